# revision 31
# baseline (speedup 1.0000x reference)
"""Trainium2 Bass kernel for attention-pooling (AttLayer).

Computes, per batch row b:
    z   = x[b] @ W + bias            # [S, A]
    t   = tanh(z)
    sc  = t @ u                      # [S]
    e   = exp(sc) * mask[b]
    out = (x[b]^T @ e) / (sum(e) + 1e-7)   # [D]

Sharding: data-parallel over batch across 8 NeuronCores (8 rows each).

Design notes:
- Host gathers only unmasked positions per row, zero-padded to S_c
  (multiple of 128). Pad columns are exactly zero, so their score is the
  constant C = u . tanh(b); the host subtracts n_pad*exp(C) from the
  denominator, which removes the mask from the device entirely.
- x is streamed twice: fp8-e3m4 transposed [D, S_c] for the score
  matmuls (W is fp8-e3m4 scaled by 32; 1/32 folded into the tanh
  activation scale), and bf16 natural [S_c, D] for the weighted sum.
- Sequence slots use the interleaved convention s' = p*NCOL + j so the
  score row [1, S_c] -> [128, NCOL] transpose is a single contiguous
  SBUF->SBUF DMA and the xn load reads NCOL*D contiguous bytes per
  partition. Stage-1 matmuls are K-contiguous (the d-chunk matmuls of
  one (ac, blk) group accumulate back-to-back into one PSUM bank) in
  near-equal <=512-col blocks; the PE streams at its issue limit.
- A warmup burst of matmuls on the preloaded W tile runs during the
  first x DMA so the PE's HAM clock gate opens before real work starts.
- Scores leave PSUM via DVE row casts (bf16); exp runs on the
  transposed [128, NCOL] tile with accum_out producing the denominator
  partial sums. The LAST batch instead computes scores in column form
  on the then-idle PE (tt chunks stationary, u moving; exp reads the
  f32 PSUM columns), removing the cast + transpose DMA from the tail
  and keeping the clock gate open; its xn uses the matching j*128+p
  slot convention. Weighted sum uses 4-way column-tiled M=1 matmuls;
  the host adds the 4 PSUM rows and divides.
"""

import math
import os
import numpy as np
import ml_dtypes

B, S, D, A = 64, 2048, 512, 256
NCORES = 8
BL = B // NCORES          # batches per core
EPS = 1e-7
WSCALE = 32.0

_cache = {}
last_results = None       # BassKernelResults of the most recent run


def _blocks_of(S_c):
    """Split S_c into near-equal seq blocks of at most 512 (multiples of
    128). Equal sizes keep every matmul in the efficient streaming regime
    instead of leaving a latency-dominated 128-col remainder block."""
    ncol = S_c // 128
    nblk = -(-S_c // 512)
    lo, rem = divmod(ncol, nblk)
    return [(lo + (1 if i < rem else 0)) * 128 for i in range(nblk)]


def _build_bass(S_c):
    import concourse.mybir as mybir
    import concourse.tile as tile
    from concourse import bacc, bass_isa

    f32 = mybir.dt.float32
    bf16 = mybir.dt.bfloat16
    fp8 = mybir.dt.float8e3
    AF = mybir.ActivationFunctionType

    assert S_c % 128 == 0
    NCOL = S_c // 128          # 128-column groups
    blocks = _blocks_of(S_c)   # e.g. [512, 512, 128]
    NBLK = len(blocks)
    starts = [sum(blocks[:i]) for i in range(NBLK)]

    NAC = A // 128
    NDC = D // 128

    nc = bacc.Bacc()

    xt = nc.declare_dram_parameter("xt", [BL, D, S_c], fp8, isOutput=False)
    xn = nc.declare_dram_parameter("xn", [BL, S_c, D], bf16, isOutput=False)
    w = nc.declare_dram_parameter("w", [D, A], fp8, isOutput=False)
    u2 = nc.declare_dram_parameter("u2", [128, NAC], bf16, isOutput=False)
    uf = nc.declare_dram_parameter("uf", [128, NAC], f32, isOutput=False)
    b2 = nc.declare_dram_parameter("b2", [128, NAC], f32, isOutput=False)
    num = nc.declare_dram_parameter("num", [4, BL, D], f32, isOutput=True)
    den = nc.declare_dram_parameter("den", [128, BL], f32, isOutput=True)

    with tile.TileContext(nc) as tc:
        with (
            tc.tile_pool(name="consts", bufs=1) as consts,
            tc.tile_pool(name="xtp", bufs=3) as xtp,
            tc.tile_pool(name="xnp", bufs=3) as xnp,
            tc.tile_pool(name="tts", bufs=8) as tts,
            tc.tile_pool(name="scp", bufs=3) as scp,
            tc.tile_pool(name="esbp", bufs=3) as esbp,
            tc.tile_pool(name="pt", bufs=4, space="PSUM") as pt,
            tc.tile_pool(name="psc", bufs=2, space="PSUM") as psc,
            tc.tile_pool(name="pacc", bufs=2, space="PSUM") as pacc,
        ):
            # --- constants, loaded once ---
            w_sb = consts.tile([128, NDC, A], fp8)  # [p, dchunk, a]
            nc.sync.dma_start(out=w_sb, in_=w.rearrange("(c p) a -> p c a", p=128))
            u_sb = consts.tile([128, NAC], bf16)
            uf_sb = consts.tile([128, NAC], f32)
            b_sb = consts.tile([128, NAC], f32)

            den_all = consts.tile([128, BL], f32)
            o_all = consts.tile([128, BL, D], f32)

            # warmup: ~3.4us of matmuls on the W tile while the first x
            # DMA is in flight, so HAM un-throttles before real work
            ps_w = pacc.tile([128, D], f32, tag="pacc")
            for i in range(22):
                nc.tensor.matmul(
                    out=ps_w[:, :128],
                    lhsT=w_sb[:, 0, 0:128],
                    rhs=w_sb[:, i % NDC, 0:128],
                    start=True,
                    stop=True,
                )

            for bi in range(BL):
                xt_t = xtp.tile([128, NDC, S_c], fp8)
                if bi == 0:
                    h = (S_c // 256) * 128
                    xt_r = xt[bi].rearrange("(c p) s -> p c s", p=128)
                    nc.sync.dma_start(out=xt_t[:, :, :h], in_=xt_r[:, :, :h])
                    nc.sync.dma_start(out=xt_t[:, :, h:], in_=xt_r[:, :, h:])
                else:
                    nc.sync.dma_start(
                        out=xt_t, in_=xt[bi].rearrange("(c p) s -> p c s", p=128)
                    )
                xn_t = xnp.tile([128, NCOL, D], bf16)
                if bi < BL - 1:
                    # gather convention: e column j holds slots p*NCOL + j
                    nc.sync.dma_start(
                        out=xn_t, in_=xn[bi].rearrange("(p j) d -> p j d", j=NCOL)
                    )
                    if bi == 0:
                        nc.sync.dma_start(out=u_sb, in_=u2[:, :])
                        nc.sync.dma_start(out=uf_sb, in_=uf[:, :])
                        nc.sync.dma_start(out=b_sb, in_=b2[:, :])
                else:
                    # last batch computes scores in column form on the PE,
                    # which yields e column j = slots j*128 + p
                    nc.sync.dma_start(
                        out=xn_t, in_=xn[bi].rearrange("(j p) d -> p j d", p=128)
                    )

                # stage 1 (weight-major): z^T[a, s] = (32W)^T @ x^T, tanh(z/32 + b).
                # blk-outer so both a-chunk tanhs of a block complete early;
                # K-contiguous d-chunk accumulation within each PSUM bank.
                tt_all = {}
                for blk in range(NBLK):
                    for ac in range(NAC):
                        ps = pt.tile([128, 512], f32, tag="pst")
                        for dc in range(NDC):
                            nc.tensor.matmul(
                                out=ps[:, : blocks[blk]],
                                lhsT=w_sb[:, dc, ac * 128 : (ac + 1) * 128],
                                rhs=xt_t[:, dc, starts[blk] : starts[blk] + blocks[blk]],
                                start=(dc == 0),
                                stop=(dc == NDC - 1),
                            )
                        tt = tts.tile([128, 512], bf16, tag="tt")
                        nc.scalar.activation(
                            out=tt[:, : blocks[blk]],
                            in_=ps[:, : blocks[blk]],
                            func=AF.Tanh,
                            bias=b_sb[:, ac : ac + 1],
                            scale=1.0 / WSCALE,
                        )
                        tt_all[(ac, blk)] = tt

                e_sb = esbp.tile([128, NCOL], bf16, tag="esb")
                if bi < BL - 1:
                    # stage 2: sc[s] = u . t[:, s]; DVE evacuates rows as bf16
                    sc_row = scp.tile([1, S_c], bf16, tag="scrow")
                    for blk in range(NBLK):
                        ps_sc = psc.tile([1, 512], f32, tag="psc")
                        for ac in range(NAC):
                            nc.tensor.matmul(
                                out=ps_sc[:, : blocks[blk]],
                                lhsT=u_sb[:, ac : ac + 1],
                                rhs=tt_all[(ac, blk)][:, : blocks[blk]],
                                start=(ac == 0),
                                stop=(ac == NAC - 1),
                            )
                        nc.vector.tensor_copy(
                            out=sc_row[:, starts[blk] : starts[blk] + blocks[blk]],
                            in_=ps_sc[:, : blocks[blk]],
                        )

                    # transpose scores into columns: sc_cols[p, j] = sc[p*NCOL+j]
                    sc_cols = esbp.tile([128, NCOL], bf16, tag="sccols")
                    nc.sync.dma_start(out=sc_cols, in_=sc_row[0:1, :])
                    nc.scalar.activation(
                        out=e_sb,
                        in_=sc_cols,
                        func=AF.Exp,
                        accum_out=den_all[:, bi : bi + 1],
                    )
                else:
                    # last batch: the PE is otherwise draining, so compute
                    # the scores in column form directly on it (tt chunks
                    # stationary, u moving) — no cast, no transpose DMA.
                    # Column j of ps_tail is the score of s-block j.
                    ps_tail = pacc.tile([128, D], f32, tag="pacc")
                    for j in range(NCOL):
                        blk, joff = 0, j
                        while joff * 128 >= blocks[blk]:
                            joff -= blocks[blk] // 128
                            blk += 1
                        for ac in range(NAC):
                            nc.tensor.matmul(
                                out=ps_tail[:, j : j + 1],
                                lhsT=tt_all[(ac, blk)][:, joff * 128 : (joff + 1) * 128],
                                rhs=u_sb[:, ac : ac + 1],
                                start=(ac == 0),
                                stop=(ac == NAC - 1),
                            )
                    nc.scalar.activation(
                        out=e_sb,
                        in_=ps_tail[:, :NCOL],
                        func=AF.Exp,
                        accum_out=den_all[:, bi : bi + 1],
                    )

                # weighted sum: 4 column-tiled concurrent M=1 matmuls; the
                # partial sums land on psum partitions 0/32/64/96 and the
                # host adds the 4 rows after the gather.
                ps_acc = pacc.tile([128, D], f32, tag="pacc")
                last_k = {g: max(k for k in range(NCOL) if k % 4 == g) for g in range(min(4, NCOL))}
                for j in range(NCOL):
                    grp = j % 4
                    nc.tensor.matmul(
                        out=ps_acc[32 * grp : 32 * grp + 1, :],
                        lhsT=e_sb[:, j : j + 1],
                        rhs=xn_t[:, j, :],
                        start=(j < 4),
                        stop=(j == last_k[grp]),
                        tile_position=(0, 32 * grp),
                    )
                nc.vector.tensor_copy(out=o_all[:, bi, :], in_=ps_acc)

                if bi == BL - 2:
                    nc.sync.dma_start(
                        out=den[:, : BL - 1], in_=den_all[:, : BL - 1]
                    )
                    for g in range(4):
                        nc.sync.dma_start(
                            out=num[g, : BL - 1, :],
                            in_=o_all[32 * g : 32 * g + 1, : BL - 1, :],
                        )

            nc.sync.dma_start(out=den[:, BL - 1 :], in_=den_all[:, BL - 1 :])
            for g in range(4):
                nc.sync.dma_start(
                    out=num[g, BL - 1 :, :],
                    in_=o_all[32 * g : 32 * g + 1, BL - 1 :, :],
                )

    nc.finalize()
    return nc


def _get_nc(S_c):
    if S_c not in _cache:
        _cache[S_c] = _build_bass(S_c)
    return _cache[S_c]


def kernel(x, mask, W, b, u):
    global last_results
    from concourse.bass_utils import run_bass_kernel_spmd

    bf = ml_dtypes.bfloat16
    f8 = ml_dtypes.float8_e3m4
    x = np.asarray(x, dtype=np.float32)
    mask = np.asarray(mask).astype(bool)
    W32 = np.asarray(W, dtype=np.float32)
    b32 = np.asarray(b, dtype=np.float32)
    u32 = np.asarray(u, dtype=np.float32)

    counts = mask.sum(axis=1)
    maxc = int(counts.max())
    S_c = min(S, max(256, 128 * ((maxc + 127) // 128)))

    # host-side compaction: gather unmasked positions, zero-pad to S_c
    xc = np.zeros((B, S_c, D), dtype=np.float32)
    for bi in range(B):
        idx = np.flatnonzero(mask[bi])
        xc[bi, : idx.size] = x[bi, idx]

    xn_h = xc.astype(bf)                                               # [B, S_c, D]
    xt_h = np.ascontiguousarray(xc.transpose(0, 2, 1)).astype(f8)      # [B, D, S_c]
    w_h = (W32 * WSCALE).astype(f8)                                    # [D, A]
    u_h = np.ascontiguousarray(
        u32[:, 0].reshape(A // 128, 128).T
    ).astype(bf)                                                       # [128, A/128]
    b_h = np.ascontiguousarray(
        b32.reshape(A // 128, 128).T
    ).astype(np.float32)                                               # [128, A/128]
    uf_h = np.ascontiguousarray(
        u32[:, 0].reshape(A // 128, 128).T
    ).astype(np.float32)                                               # [128, A/128]

    # the device score for an all-zero pad column, replicated with the
    # same quantization (z=0 exactly; tanh in f32 -> bf16 tt; u bf16).
    # Batches 0..BL-2 per core: chunk0 via PE (f32 products), chunk1 via
    # DVE mul (bf16 products) + gpsimd reduce, sum cast to bf16.
    # Last batch per core: both chunks on the PE, exp reads f32 PSUM.
    t_pad = np.tanh(b32).astype(bf).astype(np.float32)
    u_bf = u32[:, 0].astype(bf).astype(np.float32)
    c_full = np.float32(np.dot(u_bf, t_pad))
    e_pad_a = np.exp(np.float32(c_full).astype(bf).astype(np.float32))
    e_pad_b = np.exp(c_full)
    e_pad = np.full(B, e_pad_a, dtype=np.float32)
    e_pad[BL - 1 :: BL] = e_pad_b

    nc = _get_nc(S_c)
    in_maps = []
    for c in range(NCORES):
        sl = slice(c * BL, (c + 1) * BL)
        in_maps.append(
            {
                "xt": xt_h[sl],
                "xn": xn_h[sl],
                "w": w_h,
                "u2": u_h,
                "uf": uf_h,
                "b2": b_h,
            }
        )

    try:
        res = run_bass_kernel_spmd(nc, in_maps, core_ids=list(range(NCORES)))
    except ModuleNotFoundError:
        # BASS_TRACE requested but the axon NTFF hook module is absent;
        # rerun without tracing.
        os.environ["BASS_NEVER_TRACE"] = "1"
        res = run_bass_kernel_spmd(nc, in_maps, core_ids=list(range(NCORES)))
    last_results = res

    num = np.concatenate([r["num"] for r in res.results], axis=1)      # [4, B, D]
    den_d = np.concatenate([r["den"] for r in res.results], axis=1)    # [128, B]
    n_pad = (S_c - counts).astype(np.float32)                          # [B]
    denom = den_d.sum(axis=0) - n_pad * e_pad + np.float32(EPS)
    ngrp = min(4, S_c // 128)  # psum col-groups actually written
    out = num[:ngrp].sum(axis=0) / denom[:, None]
    return out.astype(np.float32)


# revision 32
# speedup vs baseline: 1.0103x; 1.0103x over previous
"""Trainium2 Bass kernel for attention-pooling (AttLayer).

Computes, per batch row b:
    z   = x[b] @ W + bias            # [S, A]
    t   = tanh(z)
    sc  = t @ u                      # [S]
    e   = exp(sc) * mask[b]
    out = (x[b]^T @ e) / (sum(e) + 1e-7)   # [D]

Sharding: data-parallel over batch across 8 NeuronCores (8 rows each).

Design notes:
- Host gathers only unmasked positions per row, zero-padded to S_c
  (multiple of 128). Pad columns are exactly zero, so their score is the
  constant C = u . tanh(b); the host subtracts n_pad*exp(C) from the
  denominator, which removes the mask from the device entirely.
- x is streamed twice: fp8-e3m4 transposed [D, S_c] for the score
  matmuls (W is fp8-e3m4 scaled by 32; 1/32 folded into the tanh
  activation scale), and bf16 natural [S_c, D] for the weighted sum.
- Sequence slots use the interleaved convention s' = p*NCOL + j so the
  score row [1, S_c] -> [128, NCOL] transpose is a single contiguous
  SBUF->SBUF DMA and the xn load reads NCOL*D contiguous bytes per
  partition. Stage-1 matmuls are K-contiguous (the d-chunk matmuls of
  one (ac, blk) group accumulate back-to-back into one PSUM bank) in
  near-equal <=512-col blocks; the PE streams at its issue limit.
- A warmup burst of matmuls on the preloaded W tile runs during the
  first x DMA so the PE's HAM clock gate opens before real work starts.
- Scores leave PSUM via DVE row casts (bf16); exp runs on the
  transposed [128, NCOL] tile with accum_out producing the denominator
  partial sums. The LAST batch instead computes scores in column form
  on the then-idle PE (tt chunks stationary, u moving; exp reads the
  f32 PSUM columns), removing the cast + transpose DMA from the tail
  and keeping the clock gate open; its xn uses the matching j*128+p
  slot convention. Weighted sum uses 4-way column-tiled M=1 matmuls;
  the host adds the 4 PSUM rows and divides.
"""

import math
import os
import numpy as np
import ml_dtypes

B, S, D, A = 64, 2048, 512, 256
NCORES = 8
BL = B // NCORES          # batches per core
EPS = 1e-7
WSCALE = 32.0

_cache = {}
last_results = None       # BassKernelResults of the most recent run


def _blocks_of(S_c):
    """Split S_c into near-equal seq blocks of at most 512 (multiples of
    128). Equal sizes keep every matmul in the efficient streaming regime
    instead of leaving a latency-dominated 128-col remainder block."""
    ncol = S_c // 128
    nblk = -(-S_c // 512)
    lo, rem = divmod(ncol, nblk)
    return [(lo + (1 if i < rem else 0)) * 128 for i in range(nblk)]


def _build_bass(S_c):
    import concourse.mybir as mybir
    import concourse.tile as tile
    from concourse import bacc, bass_isa

    f32 = mybir.dt.float32
    bf16 = mybir.dt.bfloat16
    fp8 = mybir.dt.float8e3
    AF = mybir.ActivationFunctionType

    assert S_c % 128 == 0
    NCOL = S_c // 128          # 128-column groups
    blocks = _blocks_of(S_c)   # e.g. [512, 512, 128]
    NBLK = len(blocks)
    starts = [sum(blocks[:i]) for i in range(NBLK)]

    NAC = A // 128
    NDC = D // 128

    nc = bacc.Bacc()

    xt = nc.declare_dram_parameter("xt", [BL, D, S_c], fp8, isOutput=False)
    xn = nc.declare_dram_parameter("xn", [BL, S_c, D], bf16, isOutput=False)
    w = nc.declare_dram_parameter("w", [D, A], fp8, isOutput=False)
    u2 = nc.declare_dram_parameter("u2", [128, NAC], bf16, isOutput=False)
    uf = nc.declare_dram_parameter("uf", [128, NAC], f32, isOutput=False)
    b2 = nc.declare_dram_parameter("b2", [128, NAC], f32, isOutput=False)
    num = nc.declare_dram_parameter("num", [4, BL, D], f32, isOutput=True)
    den = nc.declare_dram_parameter("den", [128, BL], f32, isOutput=True)

    with tile.TileContext(nc) as tc:
        with (
            tc.tile_pool(name="consts", bufs=1) as consts,
            tc.tile_pool(name="xtp", bufs=3) as xtp,
            tc.tile_pool(name="xnp", bufs=3) as xnp,
            tc.tile_pool(name="tts", bufs=8) as tts,
            tc.tile_pool(name="scp", bufs=3) as scp,
            tc.tile_pool(name="esbp", bufs=3) as esbp,
            tc.tile_pool(name="pt", bufs=4, space="PSUM") as pt,
            tc.tile_pool(name="psc", bufs=2, space="PSUM") as psc,
            tc.tile_pool(name="pacc", bufs=2, space="PSUM") as pacc,
        ):
            # --- constants, loaded once ---
            w_sb = consts.tile([128, NDC, A], fp8)  # [p, dchunk, a]
            nc.sync.dma_start(out=w_sb, in_=w.rearrange("(c p) a -> p c a", p=128))
            u_sb = consts.tile([128, NAC], bf16)
            uf_sb = consts.tile([128, NAC], f32)
            b_sb = consts.tile([128, NAC], f32)

            den_all = consts.tile([128, BL], f32)
            o_all = consts.tile([128, BL, D], f32)

            # warmup: ~3.4us of matmuls on the W tile while the first x
            # DMA is in flight, so HAM un-throttles before real work
            ps_w = pacc.tile([128, D], f32, tag="pacc")
            for i in range(22):
                nc.tensor.matmul(
                    out=ps_w[:, :128],
                    lhsT=w_sb[:, 0, 0:128],
                    rhs=w_sb[:, i % NDC, 0:128],
                    start=True,
                    stop=True,
                )

            for bi in range(BL):
                xt_t = xtp.tile([128, NDC, S_c], fp8)
                if bi == 0:
                    h = (S_c // 256) * 128
                    xt_r = xt[bi].rearrange("(c p) s -> p c s", p=128)
                    nc.sync.dma_start(out=xt_t[:, :, :h], in_=xt_r[:, :, :h])
                    nc.sync.dma_start(out=xt_t[:, :, h:], in_=xt_r[:, :, h:])
                else:
                    nc.sync.dma_start(
                        out=xt_t, in_=xt[bi].rearrange("(c p) s -> p c s", p=128)
                    )
                if bi == 0:
                    nc.sync.dma_start(out=b_sb, in_=b2[:, :])
                    nc.sync.dma_start(out=u_sb, in_=u2[:, :])
                    nc.sync.dma_start(out=uf_sb, in_=uf[:, :])
                xn_t = xnp.tile([128, NCOL, D], bf16)
                if bi < BL - 1:
                    # gather convention: e column j holds slots p*NCOL + j
                    nc.sync.dma_start(
                        out=xn_t, in_=xn[bi].rearrange("(p j) d -> p j d", j=NCOL)
                    )
                else:
                    # last batch computes scores in column form on the PE,
                    # which yields e column j = slots j*128 + p
                    nc.sync.dma_start(
                        out=xn_t, in_=xn[bi].rearrange("(j p) d -> p j d", p=128)
                    )
                    # early partial output DMAs: only wait on batches
                    # 0..BL-2 (program order), and nothing queues behind
                    # them except the final output DMAs
                    nc.sync.dma_start(
                        out=den[:, : BL - 1], in_=den_all[:, : BL - 1]
                    )
                    for g in range(4):
                        nc.sync.dma_start(
                            out=num[g, : BL - 1, :],
                            in_=o_all[32 * g : 32 * g + 1, : BL - 1, :],
                        )

                # stage 1 (weight-major): z^T[a, s] = (32W)^T @ x^T, tanh(z/32 + b).
                # blk-outer so both a-chunk tanhs of a block complete early;
                # K-contiguous d-chunk accumulation within each PSUM bank.
                tt_all = {}
                for blk in range(NBLK):
                    for ac in range(NAC):
                        ps = pt.tile([128, 512], f32, tag="pst")
                        for dc in range(NDC):
                            nc.tensor.matmul(
                                out=ps[:, : blocks[blk]],
                                lhsT=w_sb[:, dc, ac * 128 : (ac + 1) * 128],
                                rhs=xt_t[:, dc, starts[blk] : starts[blk] + blocks[blk]],
                                start=(dc == 0),
                                stop=(dc == NDC - 1),
                            )
                        tt = tts.tile([128, 512], bf16, tag="tt")
                        nc.scalar.activation(
                            out=tt[:, : blocks[blk]],
                            in_=ps[:, : blocks[blk]],
                            func=AF.Tanh,
                            bias=b_sb[:, ac : ac + 1],
                            scale=1.0 / WSCALE,
                        )
                        tt_all[(ac, blk)] = tt

                e_sb = esbp.tile([128, NCOL], bf16, tag="esb")
                if bi < BL - 1:
                    # stage 2: sc[s] = u . t[:, s]; DVE evacuates rows as bf16
                    sc_row = scp.tile([1, S_c], bf16, tag="scrow")
                    for blk in range(NBLK):
                        ps_sc = psc.tile([1, 512], f32, tag="psc")
                        for ac in range(NAC):
                            nc.tensor.matmul(
                                out=ps_sc[:, : blocks[blk]],
                                lhsT=u_sb[:, ac : ac + 1],
                                rhs=tt_all[(ac, blk)][:, : blocks[blk]],
                                start=(ac == 0),
                                stop=(ac == NAC - 1),
                            )
                        nc.vector.tensor_copy(
                            out=sc_row[:, starts[blk] : starts[blk] + blocks[blk]],
                            in_=ps_sc[:, : blocks[blk]],
                        )

                    # transpose scores into columns: sc_cols[p, j] = sc[p*NCOL+j]
                    sc_cols = esbp.tile([128, NCOL], bf16, tag="sccols")
                    nc.sync.dma_start(out=sc_cols, in_=sc_row[0:1, :])
                    nc.scalar.activation(
                        out=e_sb,
                        in_=sc_cols,
                        func=AF.Exp,
                        accum_out=den_all[:, bi : bi + 1],
                    )
                else:
                    # last batch: the PE is otherwise draining, so compute
                    # the scores in column form directly on it (tt chunks
                    # stationary, u moving) — no cast, no transpose DMA.
                    # Column j of ps_tail is the score of s-block j.
                    ps_tail = pacc.tile([128, D], f32, tag="pacc")
                    for j in range(NCOL):
                        blk, joff = 0, j
                        while joff * 128 >= blocks[blk]:
                            joff -= blocks[blk] // 128
                            blk += 1
                        for ac in range(NAC):
                            nc.tensor.matmul(
                                out=ps_tail[:, j : j + 1],
                                lhsT=tt_all[(ac, blk)][:, joff * 128 : (joff + 1) * 128],
                                rhs=u_sb[:, ac : ac + 1],
                                start=(ac == 0),
                                stop=(ac == NAC - 1),
                            )
                    nc.scalar.activation(
                        out=e_sb,
                        in_=ps_tail[:, :NCOL],
                        func=AF.Exp,
                        accum_out=den_all[:, bi : bi + 1],
                    )

                # weighted sum: 4 column-tiled concurrent M=1 matmuls; the
                # partial sums land on psum partitions 0/32/64/96 and the
                # host adds the 4 rows after the gather.
                ps_acc = pacc.tile([128, D], f32, tag="pacc")
                last_k = {g: max(k for k in range(NCOL) if k % 4 == g) for g in range(min(4, NCOL))}
                for j in range(NCOL):
                    grp = j % 4
                    nc.tensor.matmul(
                        out=ps_acc[32 * grp : 32 * grp + 1, :],
                        lhsT=e_sb[:, j : j + 1],
                        rhs=xn_t[:, j, :],
                        start=(j < 4),
                        stop=(j == last_k[grp]),
                        tile_position=(0, 32 * grp),
                    )
                nc.vector.tensor_copy(out=o_all[:, bi, :], in_=ps_acc)


            nc.sync.dma_start(out=den[:, BL - 1 :], in_=den_all[:, BL - 1 :])
            for g in range(4):
                nc.sync.dma_start(
                    out=num[g, BL - 1 :, :],
                    in_=o_all[32 * g : 32 * g + 1, BL - 1 :, :],
                )

    nc.finalize()
    return nc


def _get_nc(S_c):
    if S_c not in _cache:
        _cache[S_c] = _build_bass(S_c)
    return _cache[S_c]


def kernel(x, mask, W, b, u):
    global last_results
    from concourse.bass_utils import run_bass_kernel_spmd

    bf = ml_dtypes.bfloat16
    f8 = ml_dtypes.float8_e3m4
    x = np.asarray(x, dtype=np.float32)
    mask = np.asarray(mask).astype(bool)
    W32 = np.asarray(W, dtype=np.float32)
    b32 = np.asarray(b, dtype=np.float32)
    u32 = np.asarray(u, dtype=np.float32)

    counts = mask.sum(axis=1)
    maxc = int(counts.max())
    S_c = min(S, max(256, 128 * ((maxc + 127) // 128)))

    # host-side compaction: gather unmasked positions, zero-pad to S_c
    xc = np.zeros((B, S_c, D), dtype=np.float32)
    for bi in range(B):
        idx = np.flatnonzero(mask[bi])
        xc[bi, : idx.size] = x[bi, idx]

    xn_h = xc.astype(bf)                                               # [B, S_c, D]
    xt_h = np.ascontiguousarray(xc.transpose(0, 2, 1)).astype(f8)      # [B, D, S_c]
    w_h = (W32 * WSCALE).astype(f8)                                    # [D, A]
    u_h = np.ascontiguousarray(
        u32[:, 0].reshape(A // 128, 128).T
    ).astype(bf)                                                       # [128, A/128]
    b_h = np.ascontiguousarray(
        b32.reshape(A // 128, 128).T
    ).astype(np.float32)                                               # [128, A/128]
    uf_h = np.ascontiguousarray(
        u32[:, 0].reshape(A // 128, 128).T
    ).astype(np.float32)                                               # [128, A/128]

    # the device score for an all-zero pad column, replicated with the
    # same quantization (z=0 exactly; tanh in f32 -> bf16 tt; u bf16).
    # Batches 0..BL-2 per core: chunk0 via PE (f32 products), chunk1 via
    # DVE mul (bf16 products) + gpsimd reduce, sum cast to bf16.
    # Last batch per core: both chunks on the PE, exp reads f32 PSUM.
    t_pad = np.tanh(b32).astype(bf).astype(np.float32)
    u_bf = u32[:, 0].astype(bf).astype(np.float32)
    c_full = np.float32(np.dot(u_bf, t_pad))
    e_pad_a = np.exp(np.float32(c_full).astype(bf).astype(np.float32))
    e_pad_b = np.exp(c_full)
    e_pad = np.full(B, e_pad_a, dtype=np.float32)
    e_pad[BL - 1 :: BL] = e_pad_b

    nc = _get_nc(S_c)
    in_maps = []
    for c in range(NCORES):
        sl = slice(c * BL, (c + 1) * BL)
        in_maps.append(
            {
                "xt": xt_h[sl],
                "xn": xn_h[sl],
                "w": w_h,
                "u2": u_h,
                "uf": uf_h,
                "b2": b_h,
            }
        )

    try:
        res = run_bass_kernel_spmd(nc, in_maps, core_ids=list(range(NCORES)))
    except ModuleNotFoundError:
        # BASS_TRACE requested but the axon NTFF hook module is absent;
        # rerun without tracing.
        os.environ["BASS_NEVER_TRACE"] = "1"
        res = run_bass_kernel_spmd(nc, in_maps, core_ids=list(range(NCORES)))
    last_results = res

    num = np.concatenate([r["num"] for r in res.results], axis=1)      # [4, B, D]
    den_d = np.concatenate([r["den"] for r in res.results], axis=1)    # [128, B]
    n_pad = (S_c - counts).astype(np.float32)                          # [B]
    denom = den_d.sum(axis=0) - n_pad * e_pad + np.float32(EPS)
    ngrp = min(4, S_c // 128)  # psum col-groups actually written
    out = num[:ngrp].sum(axis=0) / denom[:, None]
    return out.astype(np.float32)


# revision 33
# speedup vs baseline: 1.0882x; 1.0772x over previous
"""Trainium2 Bass kernel for attention-pooling (AttLayer).

Computes, per batch row b:
    z   = x[b] @ W + bias            # [S, A]
    t   = tanh(z)
    sc  = t @ u                      # [S]
    e   = exp(sc) * mask[b]
    out = (x[b]^T @ e) / (sum(e) + 1e-7)   # [D]

Sharding: data-parallel over batch across 8 NeuronCores (8 rows each).

Design notes:
- Host gathers only unmasked positions per row, zero-padded to S_c
  (multiple of 128). Pad columns are exactly zero, so their score is the
  constant C = u . tanh(b); the host subtracts n_pad*exp(C) from the
  denominator, which removes the mask from the device entirely.
- x is streamed twice: fp8-e3m4 transposed [D, S_c] for the score
  matmuls (W is fp8-e3m4 scaled by 32; 1/32 folded into the tanh
  activation scale), and bf16 natural [S_c, D] for the weighted sum.
- Sequence slots use the interleaved convention s' = p*NCOL + j so the
  score row [1, S_c] -> [128, NCOL] transpose is a single contiguous
  SBUF->SBUF DMA and the xn load reads NCOL*D contiguous bytes per
  partition. Stage-1 matmuls are K-contiguous (the d-chunk matmuls of
  one (ac, blk) group accumulate back-to-back into one PSUM bank) in
  near-equal <=512-col blocks; the PE streams at its issue limit.
- A warmup burst of matmuls on the preloaded W tile runs during the
  first x DMA so the PE's HAM clock gate opens before real work starts.
- Scores leave PSUM via DVE row casts (bf16); exp runs on the
  transposed [128, NCOL] tile with accum_out producing the denominator
  partial sums. The LAST batch instead computes scores in column form
  on the then-idle PE (tt chunks stationary, u moving; exp reads the
  f32 PSUM columns), removing the cast + transpose DMA from the tail
  and keeping the clock gate open; its xn uses the matching j*128+p
  slot convention. Weighted sum uses 4-way column-tiled M=1 matmuls;
  the host adds the 4 PSUM rows and divides.
"""

import math
import os
import numpy as np
import ml_dtypes

B, S, D, A = 64, 2048, 512, 256
NCORES = 8
BL = B // NCORES          # batches per core
EPS = 1e-7
WSCALE = 32.0

_cache = {}
last_results = None       # BassKernelResults of the most recent run


def _blocks_of(S_c):
    """Split S_c into near-equal seq blocks of at most 512 (multiples of
    128). Equal sizes keep every matmul in the efficient streaming regime
    instead of leaving a latency-dominated 128-col remainder block."""
    ncol = S_c // 128
    nblk = -(-S_c // 512)
    lo, rem = divmod(ncol, nblk)
    return [(lo + (1 if i < rem else 0)) * 128 for i in range(nblk)]


def _build_bass(S_c):
    import concourse.mybir as mybir
    import concourse.tile as tile
    from concourse import bacc, bass_isa

    f32 = mybir.dt.float32
    bf16 = mybir.dt.bfloat16
    fp8 = mybir.dt.float8e3
    AF = mybir.ActivationFunctionType

    assert S_c % 128 == 0
    NCOL = S_c // 128          # 128-column groups
    blocks = _blocks_of(S_c)   # e.g. [512, 512, 128]
    NBLK = len(blocks)
    starts = [sum(blocks[:i]) for i in range(NBLK)]

    NAC = A // 128
    NDC = D // 128

    nc = bacc.Bacc()

    xt = nc.declare_dram_parameter("xt", [BL, D, S_c], fp8, isOutput=False)
    xn = nc.declare_dram_parameter("xn", [BL, S_c, D], bf16, isOutput=False)
    w = nc.declare_dram_parameter("w", [D, A], fp8, isOutput=False)
    u2 = nc.declare_dram_parameter("u2", [128, NAC], bf16, isOutput=False)
    uf = nc.declare_dram_parameter("uf", [128, NAC], f32, isOutput=False)
    b2 = nc.declare_dram_parameter("b2", [128, NAC], f32, isOutput=False)
    num = nc.declare_dram_parameter("num", [4, BL, D], f32, isOutput=True)
    den = nc.declare_dram_parameter("den", [128, BL], f32, isOutput=True)

    with tile.TileContext(nc) as tc:
        with (
            tc.tile_pool(name="consts", bufs=1) as consts,
            tc.tile_pool(name="xtp", bufs=3) as xtp,
            tc.tile_pool(name="xnp", bufs=3) as xnp,
            tc.tile_pool(name="tts", bufs=8) as tts,
            tc.tile_pool(name="scp", bufs=3) as scp,
            tc.tile_pool(name="esbp", bufs=3) as esbp,
            tc.tile_pool(name="pt", bufs=4, space="PSUM") as pt,
            tc.tile_pool(name="psc", bufs=2, space="PSUM") as psc,
            tc.tile_pool(name="pacc", bufs=2, space="PSUM") as pacc,
        ):
            # --- constants, loaded once ---
            w_sb = consts.tile([128, NDC, A], fp8)  # [p, dchunk, a]
            nc.sync.dma_start(out=w_sb, in_=w.rearrange("(c p) a -> p c a", p=128))
            u_sb = consts.tile([128, NAC], bf16)
            nc.sync.dma_start(out=u_sb, in_=u2[:, :])
            uf_sb = consts.tile([128, NAC], f32)
            nc.sync.dma_start(out=uf_sb, in_=uf[:, :])
            b_sb = consts.tile([128, NAC], f32)
            nc.sync.dma_start(out=b_sb, in_=b2[:, :])

            den_all = consts.tile([128, BL], f32)
            o_all = consts.tile([128, BL, D], f32)

            # warmup: ~3.4us of matmuls on the W tile while the first x
            # DMA is in flight, so HAM un-throttles before real work
            ps_w = pacc.tile([128, D], f32, tag="pacc")
            for i in range(30):
                nc.tensor.matmul(
                    out=ps_w[:, :128],
                    lhsT=w_sb[:, 0, 0:128],
                    rhs=w_sb[:, i % NDC, 0:128],
                    start=True,
                    stop=True,
                )

            for bi in range(BL):
                xt_t = xtp.tile([128, NDC, S_c], fp8)
                nc.sync.dma_start(
                    out=xt_t, in_=xt[bi].rearrange("(c p) s -> p c s", p=128)
                )
                xn_t = xnp.tile([128, NCOL, D], bf16)
                if bi < BL - 1:
                    # gather convention: e column j holds slots p*NCOL + j
                    nc.sync.dma_start(
                        out=xn_t, in_=xn[bi].rearrange("(p j) d -> p j d", j=NCOL)
                    )
                else:
                    # last batch computes scores in column form on the PE,
                    # which yields e column j = slots j*128 + p
                    nc.sync.dma_start(
                        out=xn_t, in_=xn[bi].rearrange("(j p) d -> p j d", p=128)
                    )

                # stage 1 (weight-major): z^T[a, s] = (32W)^T @ x^T, tanh(z/32 + b).
                # blk-outer so both a-chunk tanhs of a block complete early;
                # K-contiguous d-chunk accumulation within each PSUM bank.
                tt_all = {}
                for blk in range(NBLK):
                    for ac in range(NAC):
                        ps = pt.tile([128, 512], f32, tag="pst")
                        for dc in range(NDC):
                            nc.tensor.matmul(
                                out=ps[:, : blocks[blk]],
                                lhsT=w_sb[:, dc, ac * 128 : (ac + 1) * 128],
                                rhs=xt_t[:, dc, starts[blk] : starts[blk] + blocks[blk]],
                                start=(dc == 0),
                                stop=(dc == NDC - 1),
                            )
                        tt = tts.tile([128, 512], bf16, tag="tt")
                        nc.scalar.activation(
                            out=tt[:, : blocks[blk]],
                            in_=ps[:, : blocks[blk]],
                            func=AF.Tanh,
                            bias=b_sb[:, ac : ac + 1],
                            scale=1.0 / WSCALE,
                        )
                        tt_all[(ac, blk)] = tt

                e_sb = esbp.tile([128, NCOL], bf16, tag="esb")
                if bi < BL - 1:
                    # stage 2: sc[s] = u . t[:, s]; DVE evacuates rows as bf16
                    sc_row = scp.tile([1, S_c], bf16, tag="scrow")
                    for blk in range(NBLK):
                        ps_sc = psc.tile([1, 512], f32, tag="psc")
                        for ac in range(NAC):
                            nc.tensor.matmul(
                                out=ps_sc[:, : blocks[blk]],
                                lhsT=u_sb[:, ac : ac + 1],
                                rhs=tt_all[(ac, blk)][:, : blocks[blk]],
                                start=(ac == 0),
                                stop=(ac == NAC - 1),
                            )
                        nc.vector.tensor_copy(
                            out=sc_row[:, starts[blk] : starts[blk] + blocks[blk]],
                            in_=ps_sc[:, : blocks[blk]],
                        )

                    # transpose scores into columns: sc_cols[p, j] = sc[p*NCOL+j]
                    sc_cols = esbp.tile([128, NCOL], bf16, tag="sccols")
                    nc.sync.dma_start(out=sc_cols, in_=sc_row[0:1, :])
                    nc.scalar.activation(
                        out=e_sb,
                        in_=sc_cols,
                        func=AF.Exp,
                        accum_out=den_all[:, bi : bi + 1],
                    )
                else:
                    # last batch: the PE is otherwise draining, so compute
                    # the scores in column form directly on it (tt chunks
                    # stationary, u moving) — no cast, no transpose DMA.
                    # Column j of ps_tail is the score of s-block j.
                    ps_tail = pacc.tile([128, D], f32, tag="pacc")
                    for j in range(NCOL):
                        blk, joff = 0, j
                        while joff * 128 >= blocks[blk]:
                            joff -= blocks[blk] // 128
                            blk += 1
                        for ac in range(NAC):
                            nc.tensor.matmul(
                                out=ps_tail[:, j : j + 1],
                                lhsT=tt_all[(ac, blk)][:, joff * 128 : (joff + 1) * 128],
                                rhs=u_sb[:, ac : ac + 1],
                                start=(ac == 0),
                                stop=(ac == NAC - 1),
                            )
                    nc.scalar.activation(
                        out=e_sb,
                        in_=ps_tail[:, :NCOL],
                        func=AF.Exp,
                        accum_out=den_all[:, bi : bi + 1],
                    )

                # weighted sum: 4 column-tiled concurrent M=1 matmuls; the
                # partial sums land on psum partitions 0/32/64/96 and the
                # host adds the 4 rows after the gather.
                ps_acc = pacc.tile([128, D], f32, tag="pacc")
                last_k = {g: max(k for k in range(NCOL) if k % 4 == g) for g in range(min(4, NCOL))}
                for j in range(NCOL):
                    grp = j % 4
                    nc.tensor.matmul(
                        out=ps_acc[32 * grp : 32 * grp + 1, :],
                        lhsT=e_sb[:, j : j + 1],
                        rhs=xn_t[:, j, :],
                        start=(j < 4),
                        stop=(j == last_k[grp]),
                        tile_position=(0, 32 * grp),
                    )
                nc.vector.tensor_copy(out=o_all[:, bi, :], in_=ps_acc)


            nc.sync.dma_start(out=den[:, :], in_=den_all)
            for g in range(4):
                nc.sync.dma_start(
                    out=num[g], in_=o_all[32 * g : 32 * g + 1, :, :]
                )

    nc.finalize()
    return nc


def _get_nc(S_c):
    if S_c not in _cache:
        _cache[S_c] = _build_bass(S_c)
    return _cache[S_c]


def kernel(x, mask, W, b, u):
    global last_results
    from concourse.bass_utils import run_bass_kernel_spmd

    bf = ml_dtypes.bfloat16
    f8 = ml_dtypes.float8_e3m4
    x = np.asarray(x, dtype=np.float32)
    mask = np.asarray(mask).astype(bool)
    W32 = np.asarray(W, dtype=np.float32)
    b32 = np.asarray(b, dtype=np.float32)
    u32 = np.asarray(u, dtype=np.float32)

    counts = mask.sum(axis=1)
    maxc = int(counts.max())
    S_c = min(S, max(256, 128 * ((maxc + 127) // 128)))

    # host-side compaction: gather unmasked positions, zero-pad to S_c
    xc = np.zeros((B, S_c, D), dtype=np.float32)
    for bi in range(B):
        idx = np.flatnonzero(mask[bi])
        xc[bi, : idx.size] = x[bi, idx]

    xn_h = xc.astype(bf)                                               # [B, S_c, D]
    xt_h = np.ascontiguousarray(xc.transpose(0, 2, 1)).astype(f8)      # [B, D, S_c]
    w_h = (W32 * WSCALE).astype(f8)                                    # [D, A]
    u_h = np.ascontiguousarray(
        u32[:, 0].reshape(A // 128, 128).T
    ).astype(bf)                                                       # [128, A/128]
    b_h = np.ascontiguousarray(
        b32.reshape(A // 128, 128).T
    ).astype(np.float32)                                               # [128, A/128]
    uf_h = np.ascontiguousarray(
        u32[:, 0].reshape(A // 128, 128).T
    ).astype(np.float32)                                               # [128, A/128]

    # the device score for an all-zero pad column, replicated with the
    # same quantization (z=0 exactly; tanh in f32 -> bf16 tt; u bf16).
    # Batches 0..BL-2 per core: chunk0 via PE (f32 products), chunk1 via
    # DVE mul (bf16 products) + gpsimd reduce, sum cast to bf16.
    # Last batch per core: both chunks on the PE, exp reads f32 PSUM.
    t_pad = np.tanh(b32).astype(bf).astype(np.float32)
    u_bf = u32[:, 0].astype(bf).astype(np.float32)
    c_full = np.float32(np.dot(u_bf, t_pad))
    e_pad_a = np.exp(np.float32(c_full).astype(bf).astype(np.float32))
    e_pad_b = np.exp(c_full)
    e_pad = np.full(B, e_pad_a, dtype=np.float32)
    e_pad[BL - 1 :: BL] = e_pad_b

    nc = _get_nc(S_c)
    in_maps = []
    for c in range(NCORES):
        sl = slice(c * BL, (c + 1) * BL)
        in_maps.append(
            {
                "xt": xt_h[sl],
                "xn": xn_h[sl],
                "w": w_h,
                "u2": u_h,
                "uf": uf_h,
                "b2": b_h,
            }
        )

    try:
        res = run_bass_kernel_spmd(nc, in_maps, core_ids=list(range(NCORES)))
    except ModuleNotFoundError:
        # BASS_TRACE requested but the axon NTFF hook module is absent;
        # rerun without tracing.
        os.environ["BASS_NEVER_TRACE"] = "1"
        res = run_bass_kernel_spmd(nc, in_maps, core_ids=list(range(NCORES)))
    last_results = res

    num = np.concatenate([r["num"] for r in res.results], axis=1)      # [4, B, D]
    den_d = np.concatenate([r["den"] for r in res.results], axis=1)    # [128, B]
    n_pad = (S_c - counts).astype(np.float32)                          # [B]
    denom = den_d.sum(axis=0) - n_pad * e_pad + np.float32(EPS)
    ngrp = min(4, S_c // 128)  # psum col-groups actually written
    out = num[:ngrp].sum(axis=0) / denom[:, None]
    return out.astype(np.float32)
